# revision 60
# baseline (speedup 1.0000x reference)
"""Multi-head self-attention (B=2, S=2048, D=1024, H=16) on 8 TRN2 NeuronCores.

Tensor-parallel over heads: each core owns 2 heads. Accepts FULL inputs,
returns FULL output. Host pre-transposes x and slices per-head weights;
each core computes qkv -> per-head LayerNorm -> attention -> partial
output projection (over its 128 embed dims); host sums the 8 partials
and adds the projection bias.
"""

import os
import sys

import numpy as np

for _p in ("/opt/trn_rl_repo", "/root/.axon_site/_ro/trn_rl_repo"):
    if os.path.isdir(_p) and _p not in sys.path:
        sys.path.insert(0, _p)
        break

import concourse.bass as bass  # noqa: E402
import concourse.bacc as bacc  # noqa: E402
import concourse.tile as tile  # noqa: E402
from concourse import mybir  # noqa: E402
from concourse.bass_utils import run_bass_kernel_spmd  # noqa: E402

F32 = mybir.dt.float32
F32R = mybir.dt.float32r
BF16 = mybir.dt.bfloat16
AF = mybir.ActivationFunctionType
ALU = mybir.AluOpType

NCORES = 8
D = 1024
H = 16
HD = 64
HPC = H // NCORES          # heads per core = 2
DPC = HPC * HD             # embed dims per core = 128
EPS = 1e-5


class _OneTableBacc(bacc.Bacc):
    """Bacc whose activation-table pass may only pick the ln+exp+identity
    set. Every ACT func this kernel uses lives in that one set, so exactly
    one table load is emitted and phase-1 LN work can interleave with the
    softmax Exp stream with no table reloads."""

    _TABLE = "natural_log_exp_and_others"

    def insert_act_table_loads(self):
        from concourse.hw_specs import get_activation_tables

        all_tables = get_activation_tables(self.m.arch)
        assert self._TABLE in all_tables, f"{self._TABLE} missing"
        keep = all_tables[self._TABLE]
        # preserve list order (set ids are positional); make my funcs
        # resolvable only via the one combined table
        tables = [
            (k, v if k == self._TABLE else v - keep)
            for k, v in all_tables.items()
        ]
        import bass_rust as _bass_rust

        _bass_rust.insert_act_table_loads(self, tables)


def build_nc(B, S, affine):
    """Build the SPMD Bass program for one core (same program, 8 cores)."""
    T = B * S                      # total token columns
    NTB = T // 128                 # 128-token blocks
    QC = S // 512                  # q-chunks per batch
    KB = S // 128                  # k-blocks per batch
    KCH = D // 128                 # contraction chunks (8)
    SCALE = 1.0 / np.sqrt(HD)

    nc = _OneTableBacc(
        "TRN2",
        target_bir_lowering=False,
        debug=False,
        enable_asserts=True,
        num_devices=NCORES,
    )

    xT = nc.dram_tensor("xT", [D, T], BF16, kind="ExternalInput").ap()
    wq = nc.dram_tensor("wt_qkv", [D, 3 * DPC], BF16, kind="ExternalInput").ap()
    bqx = nc.dram_tensor("b_qkv_x", [128, 3 * DPC], F32, kind="ExternalInput").ap()
    wp = nc.dram_tensor("wt_proj", [DPC, D], BF16, kind="ExternalInput").ap()
    if affine:
        gb = nc.dram_tensor("c_gb", [128, 4, HD], F32, kind="ExternalInput").ap()
    outp = nc.dram_tensor("outp", [T, D], BF16, kind="ExternalOutput").ap()

    from contextlib import ExitStack

    with tile.TileContext(nc) as tc, ExitStack() as stack:
        const = stack.enter_context(tc.tile_pool(name="const", bufs=1))
        persist = stack.enter_context(tc.tile_pool(name="persist", bufs=1))

        # whole x^T resident in SBUF; the first token-chunk's DMA is issued
        # before the weights so the first qkv matmuls start ASAP
        xt_all = const.tile([128, KCH, T], BF16, tag="xt")
        nc.sync.dma_start(
            out=xt_all[:, :, 0:512],
            in_=xT.rearrange("(c p) t -> p c t", p=128)[:, :, 0:512],
        )
        wq_sb = const.tile([128, KCH, 3 * DPC], BF16, tag="wq")
        for k in range(KCH):
            nc.sync.dma_start(
                out=wq_sb[:, k, :],
                in_=wq.rearrange("(c p) n -> p c n", p=128)[:, k, :],
            )
        bqx_sb = const.tile([128, 3 * DPC], F32, tag="bqx")
        nc.sync.dma_start(out=bqx_sb, in_=bqx)
        for n in range(1, T // 512):
            nc.sync.dma_start(
                out=xt_all[:, :, n * 512 : (n + 1) * 512],
                in_=xT.rearrange("(c p) t -> p c t", p=128)[
                    :, :, n * 512 : (n + 1) * 512
                ],
            )
        wp_sb = const.tile([DPC, D], BF16, tag="wp")
        nc.sync.dma_start(out=wp_sb, in_=wp)
        eps_sb = const.tile([128, 1], F32, tag="eps")
        nc.vector.memset(eps_sb, EPS)

        if affine:
            gb_sb = const.tile([128, 4, HD], F32, tag="gb")
            nc.sync.dma_start(out=gb_sb, in_=gb)

        # persistent intermediates
        qT = persist.tile([128, T], BF16, tag="qT")     # [2h*64, tok] LN'd q^T
        kT = persist.tile([128, T], BF16, tag="kT")
        vO = persist.tile([128, HPC, NTB, HD + 1], BF16, tag="vO")
        aT = persist.tile([128, T], BF16, tag="aT")     # attention out^T
        nc.vector.memset(vO[:, :, :, HD : HD + 1], 1.0)

        # ---------------- Phase 1 emitter: qkv + LayerNorm + transpose ---
        stage1 = stack.enter_context(tc.tile_pool(name="stage1", bufs=8))
        stats_pool = stack.enter_context(tc.tile_pool(name="stats", bufs=6))

        def emit_qkv(tb, ps):
            """qkv matmuls into PSUM; stage q,k to SBUF bf16 and v to vO so
            the PSUM tile frees quickly. The qkv bias rides the staging
            copies (tensor_add with the host-broadcast bias tile) instead
            of costing a PE matmul. Returns the staged q,k tile."""
            for k in range(KCH):
                nc.tensor.matmul(
                    ps,
                    lhsT=xt_all[:, k, tb * 128 : (tb + 1) * 128],
                    rhs=wq_sb[:, k, :],
                    start=(k == 0),
                    stop=(k == KCH - 1),
                )
            qksb = stage1.tile([128, 4, HD], BF16, tag="qksb")
            nc.vector.tensor_add(
                qksb,
                ps[:, 0 : 2 * DPC].rearrange("p (g d) -> p g d", d=HD),
                bqx_sb[:, 0 : 2 * DPC].rearrange("p (g d) -> p g d", d=HD),
            )
            nc.vector.tensor_add(
                vO[:, :, tb, 0:HD],
                ps[:, 2 * DPC :].rearrange("p (h d) -> p h d", d=HD),
                bqx_sb[:, 2 * DPC :].rearrange("p (h d) -> p h d", d=HD),
            )
            return qksb

        def emit_ln(tb, qksb):
            """LayerNorm stats+apply from the SBUF staging, then DMA-xbar
            transpose into qT/kT."""
            st = stats_pool.tile([128, 4, 6], F32, tag="st")
            mv = stats_pool.tile([128, 4, 2], F32, tag="mv")
            for g in range(4):
                nc.vector.bn_stats(out=st[:, g, :], in_=qksb[:, g, :])
                nc.vector.bn_aggr(out=mv[:, g, :], in_=st[:, g, :])
            # rstd = (var+eps)^-1/2 as Exp(-0.5*Ln(var+eps)) — keeps every
            # ACT func inside the single ln+exp+identity table set
            lnv = stats_pool.tile([128, 4], F32, tag="lnv")
            nc.scalar.activation(
                out=lnv, in_=mv[:, :, 1], func=AF.Ln, bias=eps_sb
            )
            rstd = stats_pool.tile([128, 4], F32, tag="rstd")
            nc.scalar.activation(out=rstd, in_=lnv, func=AF.Exp, scale=-0.5)
            # nmr = -mu * rstd (bias for the ACT-side LN apply)
            nmr = stats_pool.tile([128, 4], F32, tag="nmr")
            nc.vector.scalar_tensor_tensor(
                out=nmr,
                in0=mv[:, :, 0],
                scalar=-1.0,
                in1=rstd,
                op0=ALU.mult,
                op1=ALU.mult,
            )
            qn = stage1.tile([128, 128], BF16, tag="qn")
            kn = stage1.tile([128, 128], BF16, tag="kn")
            for g in range(4):
                dst = qn if g < 2 else kn
                dsl = dst[:, (g % 2) * HD : (g % 2 + 1) * HD]
                if g < 2:
                    # q groups on ACT: (x - mu)*rstd == x*rstd + (-mu*rstd)
                    nc.scalar.activation(
                        out=dsl,
                        in_=qksb[:, g, :],
                        func=AF.Identity,
                        scale=rstd[:, g : g + 1],
                        bias=nmr[:, g : g + 1],
                    )
                else:
                    # k groups on DVE (bf16 in/out: 2x DVE mode)
                    nc.vector.tensor_scalar(
                        out=dsl,
                        in0=qksb[:, g, :],
                        scalar1=mv[:, g, 0:1],
                        scalar2=rstd[:, g : g + 1],
                        op0=ALU.subtract,
                        op1=ALU.mult,
                    )
                if affine:
                    nc.vector.tensor_mul(dsl, dsl, gb_sb[:, 2 * (g // 2), :])
                    nc.vector.tensor_add(
                        dsl, dsl, gb_sb[:, 2 * (g // 2) + 1, :]
                    )
            ts = slice(tb * 128, (tb + 1) * 128)
            nc.sync.dma_start_transpose(out=qT[:, ts], in_=qn)
            nc.sync.dma_start_transpose(out=kT[:, ts], in_=kn)

        # Single PSUM layout for both phases: the phase-1 qkv tiles ride
        # the projection pool's banks (temporally disjoint: projections
        # only start after the last qkv block), so batch 0's attention
        # chunks can be emitted in the middle of phase 1 — the PE-bound
        # qkv stream and the ACT-bound softmax stream overlap.
        with (
            tc.tile_pool(name="epi_ps", bufs=2, space="PSUM") as epi_ps,
            tc.tile_pool(name="o_ps", bufs=1, space="PSUM") as o_ps,
            tc.tile_pool(name="sc_ps", bufs=2, space="PSUM") as sc_ps,
            tc.tile_pool(name="exps", bufs=6) as exps,
            tc.tile_pool(name="stage2", bufs=4) as stage2,
            tc.tile_pool(name="ostage", bufs=3) as ostage,
        ):
            ooms = {}
            avs = {}
            pend = {}

            def emit_attnv(ci, kb, ex):
                b, _ = divmod(ci, QC)
                gkb = b * KB + kb
                oom = ooms[ci]
                for h in range(HPC):
                    nc.tensor.matmul(
                        oom[:, h, :],
                        lhsT=vO[:, h, gkb, :],
                        rhs=ex[:, h, :],
                        start=(kb == 0),
                        stop=(kb == KB - 1),
                    )

            def attn_kbs(ci, kbs):
                b, qc = divmod(ci, QC)
                cols = slice(b * S + qc * 512, b * S + (qc + 1) * 512)
                if ci not in ooms:
                    ooms[ci] = o_ps.tile(
                        [HD + 1, HPC, 512], F32, tag="o", name="oom"
                    )
                    pend[ci] = []
                for kb in kbs:
                    gkb = b * KB + kb
                    ks = slice(gkb * 128, (gkb + 1) * 128)
                    # two heads' score matmuls live at partition bases
                    # 0/64 -> disjoint PE row groups run concurrently;
                    # one 1024-wide exp covers both heads
                    scp = sc_ps.tile(
                        [128, HPC, 512], F32, tag="s", name="scp"
                    )
                    for h in range(HPC):
                        hp = slice(h * HD, (h + 1) * HD)
                        nc.tensor.matmul(
                            scp[:, h, :],
                            lhsT=kT[hp, ks],
                            rhs=qT[hp, cols],
                            start=True,
                            stop=True,
                        )
                    ex = exps.tile(
                        [128, HPC, 512], BF16, tag="ex", name="ex"
                    )
                    nc.scalar.activation(
                        out=ex, in_=scp, func=AF.Exp, scale=SCALE
                    )
                    # lag attnv by one kb so PE never convoys behind the
                    # chunk-start oom handoff: the next scores are already
                    # issued before the first attnv can stall
                    pend[ci].append((kb, ex))
                    if len(pend[ci]) > 1:
                        emit_attnv(ci, *pend[ci].pop(0))

            def evict_chunk(ci):
                # flush lagged attnvs, then evacuate the PSUM accumulator
                # early (raw attention sums + denominator row) so the next
                # chunk's attnv can reuse the single oom buffer without
                # waiting on the normalize chain
                for kb, ex in pend.pop(ci):
                    emit_attnv(ci, kb, ex)
                oom = ooms.pop(ci)
                dn = stage2.tile([1, HPC, 512], F32, tag="dn", name="dn")
                nc.vector.tensor_copy(out=dn, in_=oom[HD : HD + 1, :, :])
                # per-head staging at matching partition offsets (the BIR
                # verifier requires SBUF operands on identical partitions;
                # only the PSUM side may shift)
                av = stage2.tile([128, 512], F32, tag="av", name="av")
                for h in range(HPC):
                    nc.vector.tensor_copy(
                        out=av[h * HD : (h + 1) * HD, :], in_=oom[0:HD, h, :]
                    )
                avs[ci] = (dn, av)

            def norm_proj_chunk(ci):
                b, qc = divmod(ci, QC)
                cols = slice(b * S + qc * 512, b * S + (qc + 1) * 512)
                dn, av = avs.pop(ci)
                # 1/denominator on DVE (fast 18-bit custom op), broadcast
                # across partitions on the idle gpsimd engine (ACT stays
                # pure-Exp: no activation-table reloads in the phase)
                rc = stage2.tile([1, HPC, 512], F32, tag="rc", name="rc")
                nc.vector.reciprocal_approx_fast(out=rc, in_=dn)
                rbs = stage2.tile(
                    [128, HPC, 512], F32, tag="rbs", name="rbs"
                )
                nc.gpsimd.partition_broadcast(rbs, rc)
                for h in range(HPC):
                    hp = slice(h * HD, (h + 1) * HD)
                    nc.vector.tensor_mul(
                        aT[hp, cols], av[hp, :], rbs[hp, h, :]
                    )
                # fused partial projection for the 4 token blocks of this
                # q-chunk; PSUM evicted to SBUF bf16 (projection bias is
                # added on the host)
                for tbl in range(4):
                    tb = ci * 4 + tbl
                    rows = slice(tb * 128, (tb + 1) * 128)
                    ob = ostage.tile([128, D], BF16, tag="ob")
                    for nn in range(D // 512):
                        pps = epi_ps.tile(
                            [128, 512], F32, tag="pps", name="pps"
                        )
                        nc.tensor.matmul(
                            pps,
                            lhsT=aT[:, rows],
                            rhs=wp_sb[:, nn * 512 : (nn + 1) * 512],
                            start=True,
                            stop=True,
                        )
                        nc.vector.tensor_copy(
                            out=ob[:, nn * 512 : (nn + 1) * 512], in_=pps
                        )
                    nc.sync.dma_start(out=outp[rows, :], in_=ob)

            def emit_p1(tb):
                ps = epi_ps.tile([128, 512], F32, tag="pps", name="qkv_ps")
                emit_ln(tb, emit_qkv(tb, ps[:, 0 : 3 * DPC]))

            # Phase 1 prologue: batch 0's 16 token blocks
            for tb in range(NTB // 2):
                emit_p1(tb)
            # Phase 1 tail interleaved with batch 0's first two attention
            # chunks (their q/k/v are complete): PE alternates qkv and
            # score/attnv matmuls while ACT runs 32 softmax Exps early.
            # Attention (and the chunk eviction) is emitted BEFORE the
            # step's phase-1 block so the eviction copies aren't queued
            # behind that block's LN work on DVE.
            for i in range(NTB // 2):
                ci, half = divmod(i, QC * 2)
                attn_kbs(ci, (2 * half, 2 * half + 1))
                if half == QC * 2 - 1:
                    evict_chunk(ci)
                emit_p1(NTB // 2 + i)

            # steady state: chunk ci's attention brackets an older chunk's
            # normalize+projection (emitted mid-chunk so its PE matmuls and
            # DVE casts drain before chunk ci's oom eviction needs the DVE
            # queue); the PSUM accumulator is evicted immediately after each
            # chunk's last attnv so the single oom buffer hands off fast
            NCHUNK = B * QC
            norm_pending = [0, 1]
            for ci in range(2, NCHUNK):
                attn_kbs(ci, range(0, KB // 2))
                norm_proj_chunk(norm_pending.pop(0))
                attn_kbs(ci, range(KB // 2, KB))
                evict_chunk(ci)
                norm_pending.append(ci)
            for ci in norm_pending:
                norm_proj_chunk(ci)

    nc.compile()
    return nc


def make_in_maps(x, w_qkv, b_qkv, w_proj, q_gamma, q_beta, k_gamma, k_beta,
                 affine):
    B, S, _ = x.shape
    T = B * S
    xT = np.ascontiguousarray(x.reshape(T, D).T)
    import ml_dtypes
    bf = ml_dtypes.bfloat16
    in_maps = []
    for c in range(NCORES):
        rs = slice(c * DPC, (c + 1) * DPC)
        w_slice = np.concatenate(
            [w_qkv[rs], w_qkv[D:2 * D][rs.start:rs.stop], w_qkv[2 * D:][rs.start:rs.stop]],
            axis=0,
        )  # [384, 1024]
        b_slice = np.concatenate(
            [b_qkv[rs], b_qkv[D:2 * D][rs.start:rs.stop], b_qkv[2 * D:][rs.start:rs.stop]]
        )[None, :]  # [1, 384]
        m = {
            "xT": xT.astype(bf),
            "wt_qkv": np.ascontiguousarray(w_slice.T).astype(bf),
            "b_qkv_x": np.ascontiguousarray(
                np.broadcast_to(b_slice, (128, 3 * DPC))
            ).astype(np.float32),
            "wt_proj": np.ascontiguousarray(w_proj[:, rs].T).astype(bf),
        }
        if affine:
            gb = np.stack([q_gamma, q_beta, k_gamma, k_beta])  # [4, 64]
            m["c_gb"] = np.ascontiguousarray(
                np.broadcast_to(gb[None], (128, 4, HD)).astype(np.float32)
            )
        in_maps.append(m)
    return in_maps


_NC_CACHE = {}

LAST_RESULTS = None


def kernel(x, w_qkv, b_qkv, w_proj, b_proj, q_gamma, q_beta, k_gamma, k_beta,
           **unused):
    global LAST_RESULTS
    x = np.asarray(x, np.float32)
    w_qkv = np.asarray(w_qkv, np.float32)
    b_qkv = np.asarray(b_qkv, np.float32)
    w_proj = np.asarray(w_proj, np.float32)
    b_proj = np.asarray(b_proj, np.float32)
    q_gamma = np.asarray(q_gamma, np.float32)
    q_beta = np.asarray(q_beta, np.float32)
    k_gamma = np.asarray(k_gamma, np.float32)
    k_beta = np.asarray(k_beta, np.float32)

    B, S, _ = x.shape
    affine = not (
        np.all(q_gamma == 1) and np.all(k_gamma == 1)
        and np.all(q_beta == 0) and np.all(k_beta == 0)
    )
    key = (B, S, affine)
    if key not in _NC_CACHE:
        _NC_CACHE[key] = build_nc(B, S, affine)
    nc = _NC_CACHE[key]

    in_maps = make_in_maps(
        x, w_qkv, b_qkv, w_proj, q_gamma, q_beta, k_gamma, k_beta, affine
    )
    trace = bool(int(os.environ.get("BASS_KERNEL_TRACE", "0")))
    res = run_bass_kernel_spmd(
        nc, in_maps, core_ids=list(range(NCORES)), trace=trace
    )
    LAST_RESULTS = res
    acc = np.zeros((B * S, D), np.float32)
    for r in res.results:
        acc += np.asarray(r["outp"], np.float32)
    acc += b_proj[None, :]
    return acc.reshape(B, S, D)


# revision 61
# speedup vs baseline: 1.0092x; 1.0092x over previous
"""Multi-head self-attention (B=2, S=2048, D=1024, H=16) on 8 TRN2 NeuronCores.

Tensor-parallel over heads: each core owns 2 heads. Accepts FULL inputs,
returns FULL output. Host pre-transposes x and slices per-head weights;
each core computes qkv -> per-head LayerNorm -> attention -> partial
output projection (over its 128 embed dims); host sums the 8 partials
and adds the projection bias.
"""

import os
import sys

import numpy as np

for _p in ("/opt/trn_rl_repo", "/root/.axon_site/_ro/trn_rl_repo"):
    if os.path.isdir(_p) and _p not in sys.path:
        sys.path.insert(0, _p)
        break

import concourse.bass as bass  # noqa: E402
import concourse.bacc as bacc  # noqa: E402
import concourse.tile as tile  # noqa: E402
from concourse import mybir  # noqa: E402
from concourse.bass_utils import run_bass_kernel_spmd  # noqa: E402

F32 = mybir.dt.float32
F32R = mybir.dt.float32r
BF16 = mybir.dt.bfloat16
AF = mybir.ActivationFunctionType
ALU = mybir.AluOpType

NCORES = 8
D = 1024
H = 16
HD = 64
HPC = H // NCORES          # heads per core = 2
DPC = HPC * HD             # embed dims per core = 128
EPS = 1e-5


class _OneTableBacc(bacc.Bacc):
    """Bacc whose activation-table pass may only pick the ln+exp+identity
    set. Every ACT func this kernel uses lives in that one set, so exactly
    one table load is emitted and phase-1 LN work can interleave with the
    softmax Exp stream with no table reloads."""

    _TABLE = "natural_log_exp_and_others"

    def insert_act_table_loads(self):
        from concourse.hw_specs import get_activation_tables

        all_tables = get_activation_tables(self.m.arch)
        assert self._TABLE in all_tables, f"{self._TABLE} missing"
        keep = all_tables[self._TABLE]
        # preserve list order (set ids are positional); make my funcs
        # resolvable only via the one combined table
        tables = [
            (k, v if k == self._TABLE else v - keep)
            for k, v in all_tables.items()
        ]
        import bass_rust as _bass_rust

        _bass_rust.insert_act_table_loads(self, tables)


def build_nc(B, S, affine):
    """Build the SPMD Bass program for one core (same program, 8 cores)."""
    T = B * S                      # total token columns
    NTB = T // 128                 # 128-token blocks
    QC = S // 512                  # q-chunks per batch
    KB = S // 128                  # k-blocks per batch
    KCH = D // 128                 # contraction chunks (8)
    SCALE = 1.0 / np.sqrt(HD)

    nc = _OneTableBacc(
        "TRN2",
        target_bir_lowering=False,
        debug=False,
        enable_asserts=True,
        num_devices=NCORES,
    )

    xT = nc.dram_tensor("xT", [D, T], BF16, kind="ExternalInput").ap()
    wq = nc.dram_tensor("wt_qkv", [D, 3 * DPC], BF16, kind="ExternalInput").ap()
    bqx = nc.dram_tensor("b_qkv_x", [128, 3 * DPC], F32, kind="ExternalInput").ap()
    wp = nc.dram_tensor("wt_proj", [DPC, D], BF16, kind="ExternalInput").ap()
    if affine:
        gb = nc.dram_tensor("c_gb", [128, 4, HD], F32, kind="ExternalInput").ap()
    outp = nc.dram_tensor("outp", [T, D], BF16, kind="ExternalOutput").ap()

    from contextlib import ExitStack

    with tile.TileContext(nc) as tc, ExitStack() as stack:
        const = stack.enter_context(tc.tile_pool(name="const", bufs=1))
        persist = stack.enter_context(tc.tile_pool(name="persist", bufs=1))

        # whole x^T resident in SBUF; the first token-chunk's DMA is issued
        # before the weights so the first qkv matmuls start ASAP
        xt_all = const.tile([128, KCH, T], BF16, tag="xt")
        nc.sync.dma_start(
            out=xt_all[:, :, 0:512],
            in_=xT.rearrange("(c p) t -> p c t", p=128)[:, :, 0:512],
        )
        wq_sb = const.tile([128, KCH, 3 * DPC], BF16, tag="wq")
        nc.sync.dma_start(
            out=wq_sb, in_=wq.rearrange("(c p) n -> p c n", p=128)
        )
        bqx_sb = const.tile([128, 3 * DPC], F32, tag="bqx")
        nc.sync.dma_start(out=bqx_sb, in_=bqx)
        for n in range(1, T // 512):
            nc.sync.dma_start(
                out=xt_all[:, :, n * 512 : (n + 1) * 512],
                in_=xT.rearrange("(c p) t -> p c t", p=128)[
                    :, :, n * 512 : (n + 1) * 512
                ],
            )
        wp_sb = const.tile([DPC, D], BF16, tag="wp")
        nc.sync.dma_start(out=wp_sb, in_=wp)
        eps_sb = const.tile([128, 1], F32, tag="eps")
        nc.vector.memset(eps_sb, EPS)

        if affine:
            gb_sb = const.tile([128, 4, HD], F32, tag="gb")
            nc.sync.dma_start(out=gb_sb, in_=gb)

        # persistent intermediates
        qT = persist.tile([128, T], BF16, tag="qT")     # [2h*64, tok] LN'd q^T
        kT = persist.tile([128, T], BF16, tag="kT")
        vO = persist.tile([128, HPC, NTB, HD + 1], BF16, tag="vO")
        aT = persist.tile([128, T], BF16, tag="aT")     # attention out^T
        nc.vector.memset(vO[:, :, :, HD : HD + 1], 1.0)

        # ---------------- Phase 1 emitter: qkv + LayerNorm + transpose ---
        stage1 = stack.enter_context(tc.tile_pool(name="stage1", bufs=8))
        stats_pool = stack.enter_context(tc.tile_pool(name="stats", bufs=6))

        def emit_qkv(tb, ps):
            """qkv matmuls into PSUM; stage q,k to SBUF bf16 and v to vO so
            the PSUM tile frees quickly. The qkv bias rides the staging
            copies (tensor_add with the host-broadcast bias tile) instead
            of costing a PE matmul. Returns the staged q,k tile."""
            for k in range(KCH):
                nc.tensor.matmul(
                    ps,
                    lhsT=xt_all[:, k, tb * 128 : (tb + 1) * 128],
                    rhs=wq_sb[:, k, :],
                    start=(k == 0),
                    stop=(k == KCH - 1),
                )
            qksb = stage1.tile([128, 4, HD], BF16, tag="qksb")
            nc.vector.tensor_add(
                qksb,
                ps[:, 0 : 2 * DPC].rearrange("p (g d) -> p g d", d=HD),
                bqx_sb[:, 0 : 2 * DPC].rearrange("p (g d) -> p g d", d=HD),
            )
            nc.vector.tensor_add(
                vO[:, :, tb, 0:HD],
                ps[:, 2 * DPC :].rearrange("p (h d) -> p h d", d=HD),
                bqx_sb[:, 2 * DPC :].rearrange("p (h d) -> p h d", d=HD),
            )
            return qksb

        def emit_ln(tb, qksb):
            """LayerNorm stats+apply from the SBUF staging, then DMA-xbar
            transpose into qT/kT."""
            st = stats_pool.tile([128, 4, 6], F32, tag="st")
            mv = stats_pool.tile([128, 4, 2], F32, tag="mv")
            for g in range(4):
                nc.vector.bn_stats(out=st[:, g, :], in_=qksb[:, g, :])
                nc.vector.bn_aggr(out=mv[:, g, :], in_=st[:, g, :])
            # rstd = (var+eps)^-1/2 as Exp(-0.5*Ln(var+eps)) — keeps every
            # ACT func inside the single ln+exp+identity table set
            lnv = stats_pool.tile([128, 4], F32, tag="lnv")
            nc.scalar.activation(
                out=lnv, in_=mv[:, :, 1], func=AF.Ln, bias=eps_sb
            )
            rstd = stats_pool.tile([128, 4], F32, tag="rstd")
            nc.scalar.activation(out=rstd, in_=lnv, func=AF.Exp, scale=-0.5)
            # nmr = -mu * rstd (bias for the ACT-side LN apply)
            nmr = stats_pool.tile([128, 4], F32, tag="nmr")
            nc.vector.scalar_tensor_tensor(
                out=nmr,
                in0=mv[:, :, 0],
                scalar=-1.0,
                in1=rstd,
                op0=ALU.mult,
                op1=ALU.mult,
            )
            qn = stage1.tile([128, 128], BF16, tag="qn")
            kn = stage1.tile([128, 128], BF16, tag="kn")
            for g in range(4):
                dst = qn if g < 2 else kn
                dsl = dst[:, (g % 2) * HD : (g % 2 + 1) * HD]
                if g < 2:
                    # q groups on ACT: (x - mu)*rstd == x*rstd + (-mu*rstd)
                    nc.scalar.activation(
                        out=dsl,
                        in_=qksb[:, g, :],
                        func=AF.Identity,
                        scale=rstd[:, g : g + 1],
                        bias=nmr[:, g : g + 1],
                    )
                else:
                    # k groups on DVE (bf16 in/out: 2x DVE mode)
                    nc.vector.tensor_scalar(
                        out=dsl,
                        in0=qksb[:, g, :],
                        scalar1=mv[:, g, 0:1],
                        scalar2=rstd[:, g : g + 1],
                        op0=ALU.subtract,
                        op1=ALU.mult,
                    )
                if affine:
                    nc.vector.tensor_mul(dsl, dsl, gb_sb[:, 2 * (g // 2), :])
                    nc.vector.tensor_add(
                        dsl, dsl, gb_sb[:, 2 * (g // 2) + 1, :]
                    )
            ts = slice(tb * 128, (tb + 1) * 128)
            nc.sync.dma_start_transpose(out=qT[:, ts], in_=qn)
            nc.sync.dma_start_transpose(out=kT[:, ts], in_=kn)

        # Single PSUM layout for both phases: the phase-1 qkv tiles ride
        # the projection pool's banks (temporally disjoint: projections
        # only start after the last qkv block), so batch 0's attention
        # chunks can be emitted in the middle of phase 1 — the PE-bound
        # qkv stream and the ACT-bound softmax stream overlap.
        with (
            tc.tile_pool(name="epi_ps", bufs=2, space="PSUM") as epi_ps,
            tc.tile_pool(name="o_ps", bufs=1, space="PSUM") as o_ps,
            tc.tile_pool(name="sc_ps", bufs=2, space="PSUM") as sc_ps,
            tc.tile_pool(name="exps", bufs=6) as exps,
            tc.tile_pool(name="stage2", bufs=4) as stage2,
            tc.tile_pool(name="ostage", bufs=3) as ostage,
        ):
            ooms = {}
            avs = {}
            pend = {}

            def emit_attnv(ci, kb, ex):
                b, _ = divmod(ci, QC)
                gkb = b * KB + kb
                oom = ooms[ci]
                for h in range(HPC):
                    nc.tensor.matmul(
                        oom[:, h, :],
                        lhsT=vO[:, h, gkb, :],
                        rhs=ex[:, h, :],
                        start=(kb == 0),
                        stop=(kb == KB - 1),
                    )

            def attn_kbs(ci, kbs):
                b, qc = divmod(ci, QC)
                cols = slice(b * S + qc * 512, b * S + (qc + 1) * 512)
                if ci not in ooms:
                    ooms[ci] = o_ps.tile(
                        [HD + 1, HPC, 512], F32, tag="o", name="oom"
                    )
                    pend[ci] = []
                for kb in kbs:
                    gkb = b * KB + kb
                    ks = slice(gkb * 128, (gkb + 1) * 128)
                    # two heads' score matmuls live at partition bases
                    # 0/64 -> disjoint PE row groups run concurrently;
                    # one 1024-wide exp covers both heads
                    scp = sc_ps.tile(
                        [128, HPC, 512], F32, tag="s", name="scp"
                    )
                    for h in range(HPC):
                        hp = slice(h * HD, (h + 1) * HD)
                        nc.tensor.matmul(
                            scp[:, h, :],
                            lhsT=kT[hp, ks],
                            rhs=qT[hp, cols],
                            start=True,
                            stop=True,
                        )
                    ex = exps.tile(
                        [128, HPC, 512], BF16, tag="ex", name="ex"
                    )
                    nc.scalar.activation(
                        out=ex, in_=scp, func=AF.Exp, scale=SCALE
                    )
                    # lag attnv by one kb so PE never convoys behind the
                    # chunk-start oom handoff: the next scores are already
                    # issued before the first attnv can stall
                    pend[ci].append((kb, ex))
                    if len(pend[ci]) > 1:
                        emit_attnv(ci, *pend[ci].pop(0))

            def evict_chunk(ci):
                # flush lagged attnvs, then evacuate the PSUM accumulator
                # early (raw attention sums + denominator row) so the next
                # chunk's attnv can reuse the single oom buffer without
                # waiting on the normalize chain
                for kb, ex in pend.pop(ci):
                    emit_attnv(ci, kb, ex)
                oom = ooms.pop(ci)
                dn = stage2.tile([1, HPC, 512], F32, tag="dn", name="dn")
                nc.vector.tensor_copy(out=dn, in_=oom[HD : HD + 1, :, :])
                # per-head staging at matching partition offsets (the BIR
                # verifier requires SBUF operands on identical partitions;
                # only the PSUM side may shift)
                av = stage2.tile([128, 512], F32, tag="av", name="av")
                for h in range(HPC):
                    nc.vector.tensor_copy(
                        out=av[h * HD : (h + 1) * HD, :], in_=oom[0:HD, h, :]
                    )
                avs[ci] = (dn, av)

            def norm_proj_chunk(ci):
                b, qc = divmod(ci, QC)
                cols = slice(b * S + qc * 512, b * S + (qc + 1) * 512)
                dn, av = avs.pop(ci)
                # 1/denominator on DVE (fast 18-bit custom op), broadcast
                # across partitions on the idle gpsimd engine (ACT stays
                # pure-Exp: no activation-table reloads in the phase)
                rc = stage2.tile([1, HPC, 512], F32, tag="rc", name="rc")
                nc.vector.reciprocal_approx_fast(out=rc, in_=dn)
                rbs = stage2.tile(
                    [128, HPC, 512], F32, tag="rbs", name="rbs"
                )
                nc.gpsimd.partition_broadcast(rbs, rc)
                for h in range(HPC):
                    hp = slice(h * HD, (h + 1) * HD)
                    nc.vector.tensor_mul(
                        aT[hp, cols], av[hp, :], rbs[hp, h, :]
                    )
                # fused partial projection for the 4 token blocks of this
                # q-chunk; PSUM evicted to SBUF bf16 (projection bias is
                # added on the host)
                for tbl in range(4):
                    tb = ci * 4 + tbl
                    rows = slice(tb * 128, (tb + 1) * 128)
                    ob = ostage.tile([128, D], BF16, tag="ob")
                    for nn in range(D // 512):
                        pps = epi_ps.tile(
                            [128, 512], F32, tag="pps", name="pps"
                        )
                        nc.tensor.matmul(
                            pps,
                            lhsT=aT[:, rows],
                            rhs=wp_sb[:, nn * 512 : (nn + 1) * 512],
                            start=True,
                            stop=True,
                        )
                        nc.vector.tensor_copy(
                            out=ob[:, nn * 512 : (nn + 1) * 512], in_=pps
                        )
                    nc.sync.dma_start(out=outp[rows, :], in_=ob)

            def emit_p1(tb):
                ps = epi_ps.tile([128, 512], F32, tag="pps", name="qkv_ps")
                emit_ln(tb, emit_qkv(tb, ps[:, 0 : 3 * DPC]))

            # Phase 1 prologue: batch 0's 16 token blocks
            for tb in range(NTB // 2):
                emit_p1(tb)
            # Phase 1 tail interleaved with batch 0's first two attention
            # chunks (their q/k/v are complete): PE alternates qkv and
            # score/attnv matmuls while ACT runs 32 softmax Exps early
            for i in range(NTB // 2):
                emit_p1(NTB // 2 + i)
                ci, half = divmod(i, QC * 2)
                attn_kbs(ci, (2 * half, 2 * half + 1))
                if half == QC * 2 - 1:
                    evict_chunk(ci)

            # steady state: chunk ci's attention brackets an older chunk's
            # normalize+projection (emitted mid-chunk so its PE matmuls and
            # DVE casts drain before chunk ci's oom eviction needs the DVE
            # queue); the PSUM accumulator is evicted immediately after each
            # chunk's last attnv so the single oom buffer hands off fast
            NCHUNK = B * QC
            norm_pending = [0, 1]
            for ci in range(2, NCHUNK):
                attn_kbs(ci, range(0, KB // 2))
                norm_proj_chunk(norm_pending.pop(0))
                attn_kbs(ci, range(KB // 2, KB))
                evict_chunk(ci)
                norm_pending.append(ci)
            for ci in norm_pending:
                norm_proj_chunk(ci)

    nc.compile()
    return nc


def make_in_maps(x, w_qkv, b_qkv, w_proj, q_gamma, q_beta, k_gamma, k_beta,
                 affine):
    B, S, _ = x.shape
    T = B * S
    xT = np.ascontiguousarray(x.reshape(T, D).T)
    import ml_dtypes
    bf = ml_dtypes.bfloat16
    in_maps = []
    for c in range(NCORES):
        rs = slice(c * DPC, (c + 1) * DPC)
        w_slice = np.concatenate(
            [w_qkv[rs], w_qkv[D:2 * D][rs.start:rs.stop], w_qkv[2 * D:][rs.start:rs.stop]],
            axis=0,
        )  # [384, 1024]
        b_slice = np.concatenate(
            [b_qkv[rs], b_qkv[D:2 * D][rs.start:rs.stop], b_qkv[2 * D:][rs.start:rs.stop]]
        )[None, :]  # [1, 384]
        m = {
            "xT": xT.astype(bf),
            "wt_qkv": np.ascontiguousarray(w_slice.T).astype(bf),
            "b_qkv_x": np.ascontiguousarray(
                np.broadcast_to(b_slice, (128, 3 * DPC))
            ).astype(np.float32),
            "wt_proj": np.ascontiguousarray(w_proj[:, rs].T).astype(bf),
        }
        if affine:
            gb = np.stack([q_gamma, q_beta, k_gamma, k_beta])  # [4, 64]
            m["c_gb"] = np.ascontiguousarray(
                np.broadcast_to(gb[None], (128, 4, HD)).astype(np.float32)
            )
        in_maps.append(m)
    return in_maps


_NC_CACHE = {}

LAST_RESULTS = None


def kernel(x, w_qkv, b_qkv, w_proj, b_proj, q_gamma, q_beta, k_gamma, k_beta,
           **unused):
    global LAST_RESULTS
    x = np.asarray(x, np.float32)
    w_qkv = np.asarray(w_qkv, np.float32)
    b_qkv = np.asarray(b_qkv, np.float32)
    w_proj = np.asarray(w_proj, np.float32)
    b_proj = np.asarray(b_proj, np.float32)
    q_gamma = np.asarray(q_gamma, np.float32)
    q_beta = np.asarray(q_beta, np.float32)
    k_gamma = np.asarray(k_gamma, np.float32)
    k_beta = np.asarray(k_beta, np.float32)

    B, S, _ = x.shape
    affine = not (
        np.all(q_gamma == 1) and np.all(k_gamma == 1)
        and np.all(q_beta == 0) and np.all(k_beta == 0)
    )
    key = (B, S, affine)
    if key not in _NC_CACHE:
        _NC_CACHE[key] = build_nc(B, S, affine)
    nc = _NC_CACHE[key]

    in_maps = make_in_maps(
        x, w_qkv, b_qkv, w_proj, q_gamma, q_beta, k_gamma, k_beta, affine
    )
    trace = bool(int(os.environ.get("BASS_KERNEL_TRACE", "0")))
    res = run_bass_kernel_spmd(
        nc, in_maps, core_ids=list(range(NCORES)), trace=trace
    )
    LAST_RESULTS = res
    acc = np.zeros((B * S, D), np.float32)
    for r in res.results:
        acc += np.asarray(r["outp"], np.float32)
    acc += b_proj[None, :]
    return acc.reshape(B, S, D)


# revision 65
# speedup vs baseline: 1.0127x; 1.0034x over previous
"""Multi-head self-attention (B=2, S=2048, D=1024, H=16) on 8 TRN2 NeuronCores.

Tensor-parallel over heads: each core owns 2 heads. Accepts FULL inputs,
returns FULL output. Host pre-transposes x and slices per-head weights;
each core computes qkv -> per-head LayerNorm -> attention -> partial
output projection (over its 128 embed dims); host sums the 8 partials
and adds the projection bias.
"""

import os
import sys

import numpy as np

for _p in ("/opt/trn_rl_repo", "/root/.axon_site/_ro/trn_rl_repo"):
    if os.path.isdir(_p) and _p not in sys.path:
        sys.path.insert(0, _p)
        break

import concourse.bass as bass  # noqa: E402
import concourse.bacc as bacc  # noqa: E402
import concourse.tile as tile  # noqa: E402
from concourse import mybir  # noqa: E402
from concourse.bass_utils import run_bass_kernel_spmd  # noqa: E402

F32 = mybir.dt.float32
F32R = mybir.dt.float32r
BF16 = mybir.dt.bfloat16
AF = mybir.ActivationFunctionType
ALU = mybir.AluOpType

NCORES = 8
D = 1024
H = 16
HD = 64
HPC = H // NCORES          # heads per core = 2
DPC = HPC * HD             # embed dims per core = 128
EPS = 1e-5


class _OneTableBacc(bacc.Bacc):
    """Bacc whose activation-table pass may only pick the ln+exp+identity
    set. Every ACT func this kernel uses lives in that one set, so exactly
    one table load is emitted and phase-1 LN work can interleave with the
    softmax Exp stream with no table reloads."""

    _TABLE = "natural_log_exp_and_others"

    def insert_act_table_loads(self):
        from concourse.hw_specs import get_activation_tables

        all_tables = get_activation_tables(self.m.arch)
        assert self._TABLE in all_tables, f"{self._TABLE} missing"
        keep = all_tables[self._TABLE]
        # preserve list order (set ids are positional); make my funcs
        # resolvable only via the one combined table
        tables = [
            (k, v if k == self._TABLE else v - keep)
            for k, v in all_tables.items()
        ]
        import bass_rust as _bass_rust

        _bass_rust.insert_act_table_loads(self, tables)


def build_nc(B, S, affine):
    """Build the SPMD Bass program for one core (same program, 8 cores)."""
    T = B * S                      # total token columns
    NTB = T // 128                 # 128-token blocks
    QC = S // 512                  # q-chunks per batch
    KB = S // 128                  # k-blocks per batch
    KCH = D // 128                 # contraction chunks (8)
    SCALE = 1.0 / np.sqrt(HD)

    nc = _OneTableBacc(
        "TRN2",
        target_bir_lowering=False,
        debug=False,
        enable_asserts=True,
        num_devices=NCORES,
    )

    xT = nc.dram_tensor("xT", [D, T], BF16, kind="ExternalInput").ap()
    wq = nc.dram_tensor("wt_qkv", [D, 3 * DPC], BF16, kind="ExternalInput").ap()
    bqx = nc.dram_tensor("b_qkv_x", [128, 3 * DPC], F32, kind="ExternalInput").ap()
    wp = nc.dram_tensor("wt_proj", [DPC, D], BF16, kind="ExternalInput").ap()
    if affine:
        gb = nc.dram_tensor("c_gb", [128, 4, HD], F32, kind="ExternalInput").ap()
    outp = nc.dram_tensor("outp", [T, D], BF16, kind="ExternalOutput").ap()

    from contextlib import ExitStack

    with tile.TileContext(nc) as tc, ExitStack() as stack:
        const = stack.enter_context(tc.tile_pool(name="const", bufs=1))
        persist = stack.enter_context(tc.tile_pool(name="persist", bufs=1))

        # whole x^T resident in SBUF; the first token-chunk's DMA is issued
        # before the weights so the first qkv matmuls start ASAP
        xt_all = const.tile([128, KCH, T], BF16, tag="xt")
        nc.sync.dma_start(
            out=xt_all[:, :, 0:512],
            in_=xT.rearrange("(c p) t -> p c t", p=128)[:, :, 0:512],
        )
        wq_sb = const.tile([128, KCH, 3 * DPC], BF16, tag="wq")
        nc.sync.dma_start(
            out=wq_sb, in_=wq.rearrange("(c p) n -> p c n", p=128)
        )
        bqx_sb = const.tile([128, 3 * DPC], F32, tag="bqx")
        nc.sync.dma_start(out=bqx_sb, in_=bqx)
        for n in range(1, T // 512):
            nc.sync.dma_start(
                out=xt_all[:, :, n * 512 : (n + 1) * 512],
                in_=xT.rearrange("(c p) t -> p c t", p=128)[
                    :, :, n * 512 : (n + 1) * 512
                ],
            )
        wp_sb = const.tile([DPC, D], BF16, tag="wp")
        nc.sync.dma_start(out=wp_sb, in_=wp)
        eps_sb = const.tile([128, 1], F32, tag="eps")
        nc.vector.memset(eps_sb, EPS)

        if affine:
            gb_sb = const.tile([128, 4, HD], F32, tag="gb")
            nc.sync.dma_start(out=gb_sb, in_=gb)

        # persistent intermediates
        qT = persist.tile([128, T], BF16, tag="qT")     # [2h*64, tok] LN'd q^T
        kT = persist.tile([128, T], BF16, tag="kT")
        vO = persist.tile([128, HPC, NTB, HD + 1], BF16, tag="vO")
        aT = persist.tile([128, T], BF16, tag="aT")     # attention out^T
        nc.vector.memset(vO[:, :, :, HD : HD + 1], 1.0)

        # ---------------- Phase 1 emitter: qkv + LayerNorm + transpose ---
        stage1 = stack.enter_context(tc.tile_pool(name="stage1", bufs=8))
        stats_pool = stack.enter_context(tc.tile_pool(name="stats", bufs=6))

        def emit_qkv(tb, ps):
            """qkv matmuls into PSUM; stage q,k to SBUF bf16 and v to vO so
            the PSUM tile frees quickly. The qkv bias rides the staging
            copies (tensor_add with the host-broadcast bias tile) instead
            of costing a PE matmul. Returns the staged q,k tile."""
            for k in range(KCH):
                nc.tensor.matmul(
                    ps,
                    lhsT=xt_all[:, k, tb * 128 : (tb + 1) * 128],
                    rhs=wq_sb[:, k, :],
                    start=(k == 0),
                    stop=(k == KCH - 1),
                )
            qksb = stage1.tile([128, 4, HD], BF16, tag="qksb")
            nc.vector.tensor_add(
                qksb,
                ps[:, 0 : 2 * DPC].rearrange("p (g d) -> p g d", d=HD),
                bqx_sb[:, 0 : 2 * DPC].rearrange("p (g d) -> p g d", d=HD),
            )
            nc.vector.tensor_add(
                vO[:, :, tb, 0:HD],
                ps[:, 2 * DPC :].rearrange("p (h d) -> p h d", d=HD),
                bqx_sb[:, 2 * DPC :].rearrange("p (h d) -> p h d", d=HD),
            )
            return qksb

        def emit_ln(tb, qksb):
            """LayerNorm stats+apply from the SBUF staging, then DMA-xbar
            transpose into qT/kT."""
            st = stats_pool.tile([128, 4, 6], F32, tag="st")
            mv = stats_pool.tile([128, 4, 2], F32, tag="mv")
            for g in range(4):
                nc.vector.bn_stats(out=st[:, g, :], in_=qksb[:, g, :])
                nc.vector.bn_aggr(out=mv[:, g, :], in_=st[:, g, :])
            # rstd = (var+eps)^-1/2 as Exp(-0.5*Ln(var+eps)) — keeps every
            # ACT func inside the single ln+exp+identity table set
            lnv = stats_pool.tile([128, 4], F32, tag="lnv")
            nc.scalar.activation(
                out=lnv, in_=mv[:, :, 1], func=AF.Ln, bias=eps_sb
            )
            rstd = stats_pool.tile([128, 4], F32, tag="rstd")
            nc.scalar.activation(out=rstd, in_=lnv, func=AF.Exp, scale=-0.5)
            # nmr = -mu * rstd (bias for the ACT-side LN apply)
            nmr = stats_pool.tile([128, 4], F32, tag="nmr")
            nc.vector.scalar_tensor_tensor(
                out=nmr,
                in0=mv[:, :, 0],
                scalar=-1.0,
                in1=rstd,
                op0=ALU.mult,
                op1=ALU.mult,
            )
            qn = stage1.tile([128, 128], BF16, tag="qn")
            kn = stage1.tile([128, 128], BF16, tag="kn")
            for g in range(4):
                dst = qn if g < 2 else kn
                dsl = dst[:, (g % 2) * HD : (g % 2 + 1) * HD]
                if g < 2:
                    # q groups on ACT: (x - mu)*rstd == x*rstd + (-mu*rstd)
                    nc.scalar.activation(
                        out=dsl,
                        in_=qksb[:, g, :],
                        func=AF.Identity,
                        scale=rstd[:, g : g + 1],
                        bias=nmr[:, g : g + 1],
                    )
                else:
                    # k groups on DVE (bf16 in/out: 2x DVE mode)
                    nc.vector.tensor_scalar(
                        out=dsl,
                        in0=qksb[:, g, :],
                        scalar1=mv[:, g, 0:1],
                        scalar2=rstd[:, g : g + 1],
                        op0=ALU.subtract,
                        op1=ALU.mult,
                    )
                if affine:
                    nc.vector.tensor_mul(dsl, dsl, gb_sb[:, 2 * (g // 2), :])
                    nc.vector.tensor_add(
                        dsl, dsl, gb_sb[:, 2 * (g // 2) + 1, :]
                    )
            ts = slice(tb * 128, (tb + 1) * 128)
            nc.sync.dma_start_transpose(out=qT[:, ts], in_=qn)
            nc.sync.dma_start_transpose(out=kT[:, ts], in_=kn)

        # Single PSUM layout for both phases: the phase-1 qkv tiles ride
        # the projection pool's banks (temporally disjoint: projections
        # only start after the last qkv block), so batch 0's attention
        # chunks can be emitted in the middle of phase 1 — the PE-bound
        # qkv stream and the ACT-bound softmax stream overlap.
        with (
            tc.tile_pool(name="epi_ps", bufs=2, space="PSUM") as epi_ps,
            tc.tile_pool(name="o_ps", bufs=1, space="PSUM") as o_ps,
            tc.tile_pool(name="sc_ps", bufs=2, space="PSUM") as sc_ps,
            tc.tile_pool(name="exps", bufs=6) as exps,
            tc.tile_pool(name="stage2", bufs=4) as stage2,
            tc.tile_pool(name="ostage", bufs=3) as ostage,
        ):
            ooms = {}
            avs = {}
            pend = {}

            def emit_attnv(ci, kb, ex):
                b, _ = divmod(ci, QC)
                gkb = b * KB + kb
                oom = ooms[ci]
                for h in range(HPC):
                    nc.tensor.matmul(
                        oom[:, h, :],
                        lhsT=vO[:, h, gkb, :],
                        rhs=ex[:, h, :],
                        start=(kb == 0),
                        stop=(kb == KB - 1),
                    )

            def attn_kbs(ci, kbs):
                b, qc = divmod(ci, QC)
                cols = slice(b * S + qc * 512, b * S + (qc + 1) * 512)
                if ci not in ooms:
                    ooms[ci] = o_ps.tile(
                        [HD + 1, HPC, 512], F32, tag="o", name="oom"
                    )
                    pend[ci] = []
                for kb in kbs:
                    gkb = b * KB + kb
                    ks = slice(gkb * 128, (gkb + 1) * 128)
                    # two heads' score matmuls live at partition bases
                    # 0/64 -> disjoint PE row groups run concurrently;
                    # one 1024-wide exp covers both heads
                    scp = sc_ps.tile(
                        [128, HPC, 512], F32, tag="s", name="scp"
                    )
                    for h in range(HPC):
                        hp = slice(h * HD, (h + 1) * HD)
                        nc.tensor.matmul(
                            scp[:, h, :],
                            lhsT=kT[hp, ks],
                            rhs=qT[hp, cols],
                            start=True,
                            stop=True,
                        )
                    ex = exps.tile(
                        [128, HPC, 512], BF16, tag="ex", name="ex"
                    )
                    nc.scalar.activation(
                        out=ex, in_=scp, func=AF.Exp, scale=SCALE
                    )
                    # lag attnv by one kb so PE never convoys behind the
                    # chunk-start oom handoff: the next scores are already
                    # issued before the first attnv can stall
                    pend[ci].append((kb, ex))
                    if len(pend[ci]) > 1:
                        emit_attnv(ci, *pend[ci].pop(0))

            def evict_chunk(ci):
                # flush lagged attnvs, then evacuate the PSUM accumulator
                # early (raw attention sums + denominator row) so the next
                # chunk's attnv can reuse the single oom buffer without
                # waiting on the normalize chain
                for kb, ex in pend.pop(ci):
                    emit_attnv(ci, kb, ex)
                oom = ooms.pop(ci)
                dn = stage2.tile([1, HPC, 512], F32, tag="dn", name="dn")
                nc.vector.tensor_copy(out=dn, in_=oom[HD : HD + 1, :, :])
                # per-head staging at matching partition offsets (the BIR
                # verifier requires SBUF operands on identical partitions;
                # only the PSUM side may shift)
                av = stage2.tile([128, 512], F32, tag="av", name="av")
                for h in range(HPC):
                    nc.vector.tensor_copy(
                        out=av[h * HD : (h + 1) * HD, :], in_=oom[0:HD, h, :]
                    )
                avs[ci] = (dn, av)

            def norm_proj_chunk(ci):
                b, qc = divmod(ci, QC)
                cols = slice(b * S + qc * 512, b * S + (qc + 1) * 512)
                dn, av = avs.pop(ci)
                # 1/denominator on DVE (fast 18-bit custom op), broadcast
                # across partitions on the idle gpsimd engine (ACT stays
                # pure-Exp: no activation-table reloads in the phase)
                rc = stage2.tile([1, HPC, 512], F32, tag="rc", name="rc")
                nc.vector.reciprocal_approx_fast(out=rc, in_=dn)
                rbs = stage2.tile(
                    [128, HPC, 512], F32, tag="rbs", name="rbs"
                )
                nc.gpsimd.partition_broadcast(rbs, rc)
                for h in range(HPC):
                    hp = slice(h * HD, (h + 1) * HD)
                    nc.vector.tensor_mul(
                        aT[hp, cols], av[hp, :], rbs[hp, h, :]
                    )
                # fused partial projection for the 4 token blocks of this
                # q-chunk; PSUM evicted to SBUF bf16 (projection bias is
                # added on the host)
                for tbl in range(4):
                    tb = ci * 4 + tbl
                    rows = slice(tb * 128, (tb + 1) * 128)
                    ob = ostage.tile([128, D], BF16, tag="ob")
                    for nn in range(D // 512):
                        pps = epi_ps.tile(
                            [128, 512], F32, tag="pps", name="pps"
                        )
                        nc.tensor.matmul(
                            pps,
                            lhsT=aT[:, rows],
                            rhs=wp_sb[:, nn * 512 : (nn + 1) * 512],
                            start=True,
                            stop=True,
                        )
                        nc.vector.tensor_copy(
                            out=ob[:, nn * 512 : (nn + 1) * 512], in_=pps
                        )
                    nc.sync.dma_start(out=outp[rows, :], in_=ob)

            def emit_p1(tb):
                ps = epi_ps.tile([128, 512], F32, tag="pps", name="qkv_ps")
                emit_ln(tb, emit_qkv(tb, ps[:, 0 : 3 * DPC]))

            # Phase 1 prologue: batch 0's 16 token blocks
            for tb in range(NTB // 2):
                emit_p1(tb)
            # Phase 1 tail interleaved with batch 0's first two attention
            # chunks (their q/k/v are complete): PE alternates qkv and
            # score/attnv matmuls while ACT runs 32 softmax Exps early
            for i in range(NTB // 2):
                emit_p1(NTB // 2 + i)
                ci, half = divmod(i, QC * 2)
                attn_kbs(ci, (2 * half, 2 * half + 1))
                if half == QC * 2 - 1:
                    evict_chunk(ci)

            # steady state: chunk ci's attention brackets an older chunk's
            # normalize+projection (emitted mid-chunk so its PE matmuls and
            # DVE casts drain before chunk ci's oom eviction needs the DVE
            # queue); the PSUM accumulator is evicted immediately after each
            # chunk's last attnv so the single oom buffer hands off fast
            NCHUNK = B * QC
            norm_pending = [0, 1]
            for ci in range(2, NCHUNK):
                attn_kbs(ci, range(0, KB // 2))
                norm_proj_chunk(norm_pending.pop(0))
                attn_kbs(ci, range(KB // 2, KB))
                evict_chunk(ci)
                norm_pending.append(ci)
            for ci in norm_pending:
                norm_proj_chunk(ci)

    nc.compile()
    return nc


def make_in_maps(x, w_qkv, b_qkv, w_proj, q_gamma, q_beta, k_gamma, k_beta,
                 affine):
    B, S, _ = x.shape
    T = B * S
    xT = np.ascontiguousarray(x.reshape(T, D).T)
    import ml_dtypes
    bf = ml_dtypes.bfloat16
    in_maps = []
    for c in range(NCORES):
        rs = slice(c * DPC, (c + 1) * DPC)
        w_slice = np.concatenate(
            [w_qkv[rs], w_qkv[D:2 * D][rs.start:rs.stop], w_qkv[2 * D:][rs.start:rs.stop]],
            axis=0,
        )  # [384, 1024]
        b_slice = np.concatenate(
            [b_qkv[rs], b_qkv[D:2 * D][rs.start:rs.stop], b_qkv[2 * D:][rs.start:rs.stop]]
        )[None, :]  # [1, 384]
        m = {
            "xT": xT.astype(bf),
            "wt_qkv": np.ascontiguousarray(w_slice.T).astype(bf),
            "b_qkv_x": np.ascontiguousarray(
                np.broadcast_to(b_slice, (128, 3 * DPC))
            ).astype(np.float32),
            "wt_proj": np.ascontiguousarray(w_proj[:, rs].T).astype(bf),
        }
        if affine:
            gb = np.stack([q_gamma, q_beta, k_gamma, k_beta])  # [4, 64]
            m["c_gb"] = np.ascontiguousarray(
                np.broadcast_to(gb[None], (128, 4, HD)).astype(np.float32)
            )
        in_maps.append(m)
    return in_maps


_NC_CACHE = {}

LAST_RESULTS = None


def kernel(x, w_qkv, b_qkv, w_proj, b_proj, q_gamma, q_beta, k_gamma, k_beta,
           **unused):
    global LAST_RESULTS
    x = np.asarray(x, np.float32)
    w_qkv = np.asarray(w_qkv, np.float32)
    b_qkv = np.asarray(b_qkv, np.float32)
    w_proj = np.asarray(w_proj, np.float32)
    b_proj = np.asarray(b_proj, np.float32)
    q_gamma = np.asarray(q_gamma, np.float32)
    q_beta = np.asarray(q_beta, np.float32)
    k_gamma = np.asarray(k_gamma, np.float32)
    k_beta = np.asarray(k_beta, np.float32)

    B, S, _ = x.shape
    affine = not (
        np.all(q_gamma == 1) and np.all(k_gamma == 1)
        and np.all(q_beta == 0) and np.all(k_beta == 0)
    )
    key = (B, S, affine)
    if key not in _NC_CACHE:
        _NC_CACHE[key] = build_nc(B, S, affine)
    nc = _NC_CACHE[key]

    in_maps = make_in_maps(
        x, w_qkv, b_qkv, w_proj, q_gamma, q_beta, k_gamma, k_beta, affine
    )
    trace = bool(int(os.environ.get("BASS_KERNEL_TRACE", "0")))
    res = run_bass_kernel_spmd(
        nc, in_maps, core_ids=list(range(NCORES)), trace=trace
    )
    LAST_RESULTS = res
    acc = np.zeros((B * S, D), np.float32)
    for r in res.results:
        acc += np.asarray(r["outp"], np.float32)
    acc += b_proj[None, :]
    return acc.reshape(B, S, D)


# revision 67
# speedup vs baseline: 1.0169x; 1.0042x over previous
"""Multi-head self-attention (B=2, S=2048, D=1024, H=16) on 8 TRN2 NeuronCores.

Tensor-parallel over heads: each core owns 2 heads. Accepts FULL inputs,
returns FULL output. Host pre-transposes x and slices per-head weights;
each core computes qkv -> per-head LayerNorm -> attention -> partial
output projection (over its 128 embed dims); host sums the 8 partials
and adds the projection bias.
"""

import os
import sys

import numpy as np

for _p in ("/opt/trn_rl_repo", "/root/.axon_site/_ro/trn_rl_repo"):
    if os.path.isdir(_p) and _p not in sys.path:
        sys.path.insert(0, _p)
        break

import concourse.bass as bass  # noqa: E402
import concourse.bacc as bacc  # noqa: E402
import concourse.tile as tile  # noqa: E402
from concourse import mybir  # noqa: E402
from concourse.bass_utils import run_bass_kernel_spmd  # noqa: E402

F32 = mybir.dt.float32
F32R = mybir.dt.float32r
BF16 = mybir.dt.bfloat16
AF = mybir.ActivationFunctionType
ALU = mybir.AluOpType

NCORES = 8
D = 1024
H = 16
HD = 64
HPC = H // NCORES          # heads per core = 2
DPC = HPC * HD             # embed dims per core = 128
EPS = 1e-5


class _OneTableBacc(bacc.Bacc):
    """Bacc whose activation-table pass may only pick the ln+exp+identity
    set. Every ACT func this kernel uses lives in that one set, so exactly
    one table load is emitted and phase-1 LN work can interleave with the
    softmax Exp stream with no table reloads."""

    _TABLE = "natural_log_exp_and_others"

    def insert_act_table_loads(self):
        from concourse.hw_specs import get_activation_tables

        all_tables = get_activation_tables(self.m.arch)
        assert self._TABLE in all_tables, f"{self._TABLE} missing"
        keep = all_tables[self._TABLE]
        # preserve list order (set ids are positional); make my funcs
        # resolvable only via the one combined table
        tables = [
            (k, v if k == self._TABLE else v - keep)
            for k, v in all_tables.items()
        ]
        import bass_rust as _bass_rust

        _bass_rust.insert_act_table_loads(self, tables)


def build_nc(B, S, affine):
    """Build the SPMD Bass program for one core (same program, 8 cores)."""
    T = B * S                      # total token columns
    NTB = T // 128                 # 128-token blocks
    QC = S // 512                  # q-chunks per batch
    KB = S // 128                  # k-blocks per batch
    KCH = D // 128                 # contraction chunks (8)
    SCALE = 1.0 / np.sqrt(HD)

    nc = _OneTableBacc(
        "TRN2",
        target_bir_lowering=False,
        debug=False,
        enable_asserts=True,
        num_devices=NCORES,
    )

    xT = nc.dram_tensor("xT", [D, T], BF16, kind="ExternalInput").ap()
    wq = nc.dram_tensor("wt_qkv", [D, 3 * DPC], BF16, kind="ExternalInput").ap()
    bqx = nc.dram_tensor("b_qkv_x", [128, 3 * DPC], F32, kind="ExternalInput").ap()
    wp = nc.dram_tensor("wt_proj", [DPC, D], BF16, kind="ExternalInput").ap()
    if affine:
        gb = nc.dram_tensor("c_gb", [128, 4, HD], F32, kind="ExternalInput").ap()
    outp = nc.dram_tensor("outp", [T, D], BF16, kind="ExternalOutput").ap()

    from contextlib import ExitStack

    with tile.TileContext(nc) as tc, ExitStack() as stack:
        const = stack.enter_context(tc.tile_pool(name="const", bufs=1))
        persist = stack.enter_context(tc.tile_pool(name="persist", bufs=1))

        # whole x^T resident in SBUF; the first token-chunk's DMA is issued
        # before the weights so the first qkv matmuls start ASAP
        xt_all = const.tile([128, KCH, T], BF16, tag="xt")
        nc.sync.dma_start(
            out=xt_all[:, :, 0:512],
            in_=xT.rearrange("(c p) t -> p c t", p=128)[:, :, 0:512],
        )
        wq_sb = const.tile([128, KCH, 3 * DPC], BF16, tag="wq")
        nc.sync.dma_start(
            out=wq_sb, in_=wq.rearrange("(c p) n -> p c n", p=128)
        )
        bqx_sb = const.tile([128, 3 * DPC], F32, tag="bqx")
        nc.sync.dma_start(out=bqx_sb, in_=bqx)
        for n in range(1, T // 512):
            nc.sync.dma_start(
                out=xt_all[:, :, n * 512 : (n + 1) * 512],
                in_=xT.rearrange("(c p) t -> p c t", p=128)[
                    :, :, n * 512 : (n + 1) * 512
                ],
            )
        wp_sb = const.tile([DPC, D], BF16, tag="wp")
        nc.sync.dma_start(out=wp_sb, in_=wp)
        eps_sb = const.tile([128, 1], F32, tag="eps")
        nc.vector.memset(eps_sb, EPS)

        if affine:
            gb_sb = const.tile([128, 4, HD], F32, tag="gb")
            nc.sync.dma_start(out=gb_sb, in_=gb)

        # persistent intermediates
        qT = persist.tile([128, T], BF16, tag="qT")     # [2h*64, tok] LN'd q^T
        kT = persist.tile([128, T], BF16, tag="kT")
        vO = persist.tile([128, HPC, NTB, HD + 1], BF16, tag="vO")
        aT = persist.tile([128, T], BF16, tag="aT")     # attention out^T
        nc.vector.memset(vO[:, :, :, HD : HD + 1], 1.0)

        # ---------------- Phase 1 emitter: qkv + LayerNorm + transpose ---
        stage1 = stack.enter_context(tc.tile_pool(name="stage1", bufs=8))
        stats_pool = stack.enter_context(tc.tile_pool(name="stats", bufs=6))

        def emit_qkv(tb, ps):
            """qkv matmuls into PSUM; stage q,k to SBUF bf16 and v to vO so
            the PSUM tile frees quickly. The qkv bias rides the staging
            copies (tensor_add with the host-broadcast bias tile) instead
            of costing a PE matmul. Returns the staged q,k tile."""
            for k in range(KCH):
                nc.tensor.matmul(
                    ps,
                    lhsT=xt_all[:, k, tb * 128 : (tb + 1) * 128],
                    rhs=wq_sb[:, k, :],
                    start=(k == 0),
                    stop=(k == KCH - 1),
                )
            qksb = stage1.tile([128, 4, HD], BF16, tag="qksb")
            nc.vector.tensor_add(
                qksb,
                ps[:, 0 : 2 * DPC].rearrange("p (g d) -> p g d", d=HD),
                bqx_sb[:, 0 : 2 * DPC].rearrange("p (g d) -> p g d", d=HD),
            )
            nc.vector.tensor_add(
                vO[:, :, tb, 0:HD],
                ps[:, 2 * DPC :].rearrange("p (h d) -> p h d", d=HD),
                bqx_sb[:, 2 * DPC :].rearrange("p (h d) -> p h d", d=HD),
            )
            return qksb

        def emit_ln(tb, qksb):
            """LayerNorm stats+apply from the SBUF staging, then DMA-xbar
            transpose into qT/kT."""
            st = stats_pool.tile([128, 4, 6], F32, tag="st")
            mv = stats_pool.tile([128, 4, 2], F32, tag="mv")
            for g in range(4):
                nc.vector.bn_stats(out=st[:, g, :], in_=qksb[:, g, :])
                nc.vector.bn_aggr(out=mv[:, g, :], in_=st[:, g, :])
            # rstd = (var+eps)^-1/2 as Exp(-0.5*Ln(var+eps)) — keeps every
            # ACT func inside the single ln+exp+identity table set
            lnv = stats_pool.tile([128, 4], F32, tag="lnv")
            nc.scalar.activation(
                out=lnv, in_=mv[:, :, 1], func=AF.Ln, bias=eps_sb
            )
            rstd = stats_pool.tile([128, 4], F32, tag="rstd")
            nc.scalar.activation(out=rstd, in_=lnv, func=AF.Exp, scale=-0.5)
            # nmr = -mu * rstd (bias for the ACT-side LN apply)
            nmr = stats_pool.tile([128, 4], F32, tag="nmr")
            nc.vector.scalar_tensor_tensor(
                out=nmr,
                in0=mv[:, :, 0],
                scalar=-1.0,
                in1=rstd,
                op0=ALU.mult,
                op1=ALU.mult,
            )
            qn = stage1.tile([128, 128], BF16, tag="qn")
            kn = stage1.tile([128, 128], BF16, tag="kn")
            for g in range(4):
                dst = qn if g < 2 else kn
                dsl = dst[:, (g % 2) * HD : (g % 2 + 1) * HD]
                if g < 2:
                    # q groups on ACT: (x - mu)*rstd == x*rstd + (-mu*rstd)
                    nc.scalar.activation(
                        out=dsl,
                        in_=qksb[:, g, :],
                        func=AF.Identity,
                        scale=rstd[:, g : g + 1],
                        bias=nmr[:, g : g + 1],
                    )
                else:
                    # k groups on DVE (bf16 in/out: 2x DVE mode)
                    nc.vector.tensor_scalar(
                        out=dsl,
                        in0=qksb[:, g, :],
                        scalar1=mv[:, g, 0:1],
                        scalar2=rstd[:, g : g + 1],
                        op0=ALU.subtract,
                        op1=ALU.mult,
                    )
                if affine:
                    nc.vector.tensor_mul(dsl, dsl, gb_sb[:, 2 * (g // 2), :])
                    nc.vector.tensor_add(
                        dsl, dsl, gb_sb[:, 2 * (g // 2) + 1, :]
                    )
            ts = slice(tb * 128, (tb + 1) * 128)
            nc.sync.dma_start_transpose(out=qT[:, ts], in_=qn)
            nc.sync.dma_start_transpose(out=kT[:, ts], in_=kn)

        # Single PSUM layout for both phases: the phase-1 qkv tiles ride
        # the projection pool's banks (temporally disjoint: projections
        # only start after the last qkv block), so batch 0's attention
        # chunks can be emitted in the middle of phase 1 — the PE-bound
        # qkv stream and the ACT-bound softmax stream overlap.
        with (
            tc.tile_pool(name="epi_ps", bufs=2, space="PSUM") as epi_ps,
            tc.tile_pool(name="o_ps", bufs=1, space="PSUM") as o_ps,
            tc.tile_pool(name="sc_ps", bufs=2, space="PSUM") as sc_ps,
            tc.tile_pool(name="exps", bufs=6) as exps,
            tc.tile_pool(name="stage2", bufs=4) as stage2,
            tc.tile_pool(name="ostage", bufs=3) as ostage,
        ):
            ooms = {}
            avs = {}
            pend = {}

            def emit_attnv(ci, kb, ex):
                b, _ = divmod(ci, QC)
                gkb = b * KB + kb
                oom = ooms[ci]
                for h in range(HPC):
                    nc.tensor.matmul(
                        oom[:, h, :],
                        lhsT=vO[:, h, gkb, :],
                        rhs=ex[:, h, :],
                        start=(kb == 0),
                        stop=(kb == KB - 1),
                    )

            def attn_kbs(ci, kbs):
                b, qc = divmod(ci, QC)
                cols = slice(b * S + qc * 512, b * S + (qc + 1) * 512)
                if ci not in ooms:
                    ooms[ci] = o_ps.tile(
                        [HD + 1, HPC, 512], F32, tag="o", name="oom"
                    )
                    pend[ci] = []
                for kb in kbs:
                    gkb = b * KB + kb
                    ks = slice(gkb * 128, (gkb + 1) * 128)
                    # two heads' score matmuls live at partition bases
                    # 0/64 -> disjoint PE row groups run concurrently;
                    # one 1024-wide exp covers both heads
                    scp = sc_ps.tile(
                        [128, HPC, 512], F32, tag="s", name="scp"
                    )
                    for h in range(HPC):
                        hp = slice(h * HD, (h + 1) * HD)
                        nc.tensor.matmul(
                            scp[:, h, :],
                            lhsT=kT[hp, ks],
                            rhs=qT[hp, cols],
                            start=True,
                            stop=True,
                        )
                    ex = exps.tile(
                        [128, HPC, 512], BF16, tag="ex", name="ex"
                    )
                    nc.scalar.activation(
                        out=ex, in_=scp, func=AF.Exp, scale=SCALE
                    )
                    # lag attnv by one kb so PE never convoys behind the
                    # chunk-start oom handoff: the next scores are already
                    # issued before the first attnv can stall
                    pend[ci].append((kb, ex))
                    if len(pend[ci]) > 1:
                        emit_attnv(ci, *pend[ci].pop(0))

            def evict_chunk(ci):
                # flush lagged attnvs, then evacuate the PSUM accumulator
                # early (raw attention sums + denominator row) so the next
                # chunk's attnv can reuse the single oom buffer without
                # waiting on the normalize chain
                for kb, ex in pend.pop(ci):
                    emit_attnv(ci, kb, ex)
                oom = ooms.pop(ci)
                dn = stage2.tile([1, HPC, 512], F32, tag="dn", name="dn")
                nc.vector.tensor_copy(out=dn, in_=oom[HD : HD + 1, :, :])
                # per-head staging at matching partition offsets (the BIR
                # verifier requires SBUF operands on identical partitions;
                # only the PSUM side may shift)
                av = stage2.tile([128, 512], F32, tag="av", name="av")
                for h in range(HPC):
                    nc.vector.tensor_copy(
                        out=av[h * HD : (h + 1) * HD, :], in_=oom[0:HD, h, :]
                    )
                avs[ci] = (dn, av)

            def norm_proj_chunk(ci):
                b, qc = divmod(ci, QC)
                cols = slice(b * S + qc * 512, b * S + (qc + 1) * 512)
                dn, av = avs.pop(ci)
                # 1/denominator on DVE (fast 18-bit custom op), broadcast
                # across partitions on the idle gpsimd engine (ACT stays
                # pure-Exp: no activation-table reloads in the phase)
                rc = stage2.tile([1, HPC, 512], F32, tag="rc", name="rc")
                nc.vector.reciprocal_approx_fast(out=rc, in_=dn)
                rbs = stage2.tile(
                    [128, HPC, 512], F32, tag="rbs", name="rbs"
                )
                nc.gpsimd.partition_broadcast(rbs, rc)
                for h in range(HPC):
                    hp = slice(h * HD, (h + 1) * HD)
                    nc.vector.tensor_mul(
                        aT[hp, cols], av[hp, :], rbs[hp, h, :]
                    )
                # fused partial projection for the 4 token blocks of this
                # q-chunk; PSUM evicted to SBUF bf16 (projection bias is
                # added on the host)
                for tbl in range(4):
                    tb = ci * 4 + tbl
                    rows = slice(tb * 128, (tb + 1) * 128)
                    ob = ostage.tile([128, D], BF16, tag="ob")
                    for nn in range(D // 512):
                        pps = epi_ps.tile(
                            [128, 512], F32, tag="pps", name="pps"
                        )
                        nc.tensor.matmul(
                            pps,
                            lhsT=aT[:, rows],
                            rhs=wp_sb[:, nn * 512 : (nn + 1) * 512],
                            start=True,
                            stop=True,
                        )
                        nc.vector.tensor_copy(
                            out=ob[:, nn * 512 : (nn + 1) * 512], in_=pps
                        )
                    nc.sync.dma_start(out=outp[rows, :], in_=ob)

            def emit_p1(tb):
                ps = epi_ps.tile([128, 512], F32, tag="pps", name="qkv_ps")
                emit_ln(tb, emit_qkv(tb, ps[:, 0 : 3 * DPC]))

            # Phase 1 prologue: batch 0's 16 token blocks
            for tb in range(NTB // 2):
                emit_p1(tb)
            # Phase 1 tail interleaved with batch 0's first two attention
            # chunks (their q/k/v are complete): PE alternates qkv and
            # score/attnv matmuls while ACT runs 32 softmax Exps early
            for i in range(NTB // 2):
                emit_p1(NTB // 2 + i)
                ci, half = divmod(i, QC * 2)
                attn_kbs(ci, (2 * half, 2 * half + 1))
                if half == QC * 2 - 1:
                    evict_chunk(ci)

            # steady state: chunk ci's attention brackets an older chunk's
            # normalize+projection (emitted mid-chunk so its PE matmuls and
            # DVE casts drain before chunk ci's oom eviction needs the DVE
            # queue); the PSUM accumulator is evicted immediately after each
            # chunk's last attnv so the single oom buffer hands off fast
            NCHUNK = B * QC
            norm_pending = [0, 1]
            for ci in range(2, NCHUNK):
                attn_kbs(ci, range(0, KB // 2))
                norm_proj_chunk(norm_pending.pop(0))
                attn_kbs(ci, range(KB // 2, KB))
                evict_chunk(ci)
                norm_pending.append(ci)
            for ci in norm_pending:
                norm_proj_chunk(ci)

    nc.compile()
    return nc


def make_in_maps(x, w_qkv, b_qkv, w_proj, q_gamma, q_beta, k_gamma, k_beta,
                 affine):
    B, S, _ = x.shape
    T = B * S
    xT = np.ascontiguousarray(x.reshape(T, D).T)
    import ml_dtypes
    bf = ml_dtypes.bfloat16
    in_maps = []
    for c in range(NCORES):
        rs = slice(c * DPC, (c + 1) * DPC)
        w_slice = np.concatenate(
            [w_qkv[rs], w_qkv[D:2 * D][rs.start:rs.stop], w_qkv[2 * D:][rs.start:rs.stop]],
            axis=0,
        )  # [384, 1024]
        b_slice = np.concatenate(
            [b_qkv[rs], b_qkv[D:2 * D][rs.start:rs.stop], b_qkv[2 * D:][rs.start:rs.stop]]
        )[None, :]  # [1, 384]
        m = {
            "xT": xT.astype(bf),
            "wt_qkv": np.ascontiguousarray(w_slice.T).astype(bf),
            "b_qkv_x": np.ascontiguousarray(
                np.broadcast_to(b_slice, (128, 3 * DPC))
            ).astype(np.float32),
            "wt_proj": np.ascontiguousarray(w_proj[:, rs].T).astype(bf),
        }
        if affine:
            gb = np.stack([q_gamma, q_beta, k_gamma, k_beta])  # [4, 64]
            m["c_gb"] = np.ascontiguousarray(
                np.broadcast_to(gb[None], (128, 4, HD)).astype(np.float32)
            )
        in_maps.append(m)
    return in_maps


_NC_CACHE = {}

LAST_RESULTS = None


def kernel(x, w_qkv, b_qkv, w_proj, b_proj, q_gamma, q_beta, k_gamma, k_beta,
           **unused):
    global LAST_RESULTS
    x = np.asarray(x, np.float32)
    w_qkv = np.asarray(w_qkv, np.float32)
    b_qkv = np.asarray(b_qkv, np.float32)
    w_proj = np.asarray(w_proj, np.float32)
    b_proj = np.asarray(b_proj, np.float32)
    q_gamma = np.asarray(q_gamma, np.float32)
    q_beta = np.asarray(q_beta, np.float32)
    k_gamma = np.asarray(k_gamma, np.float32)
    k_beta = np.asarray(k_beta, np.float32)

    B, S, _ = x.shape
    affine = not (
        np.all(q_gamma == 1) and np.all(k_gamma == 1)
        and np.all(q_beta == 0) and np.all(k_beta == 0)
    )
    key = (B, S, affine)
    if key not in _NC_CACHE:
        _NC_CACHE[key] = build_nc(B, S, affine)
    nc = _NC_CACHE[key]

    in_maps = make_in_maps(
        x, w_qkv, b_qkv, w_proj, q_gamma, q_beta, k_gamma, k_beta, affine
    )
    trace = bool(int(os.environ.get("BASS_KERNEL_TRACE", "0")))
    res = run_bass_kernel_spmd(
        nc, in_maps, core_ids=list(range(NCORES)), trace=trace
    )
    LAST_RESULTS = res
    acc = np.zeros((B * S, D), np.float32)
    for r in res.results:
        acc += np.asarray(r["outp"], np.float32)
    acc += b_proj[None, :]
    return acc.reshape(B, S, D)


# revision 71
# speedup vs baseline: 1.0215x; 1.0045x over previous
"""Multi-head self-attention (B=2, S=2048, D=1024, H=16) on 8 TRN2 NeuronCores.

Tensor-parallel over heads: each core owns 2 heads. Accepts FULL inputs,
returns FULL output. Host pre-transposes x and slices per-head weights;
each core computes qkv -> per-head LayerNorm -> attention -> partial
output projection (over its 128 embed dims); host sums the 8 partials
and adds the projection bias.
"""

import os
import sys

import numpy as np

for _p in ("/opt/trn_rl_repo", "/root/.axon_site/_ro/trn_rl_repo"):
    if os.path.isdir(_p) and _p not in sys.path:
        sys.path.insert(0, _p)
        break

import concourse.bass as bass  # noqa: E402
import concourse.bacc as bacc  # noqa: E402
import concourse.tile as tile  # noqa: E402
from concourse import mybir  # noqa: E402
from concourse.bass_utils import run_bass_kernel_spmd  # noqa: E402

F32 = mybir.dt.float32
F32R = mybir.dt.float32r
BF16 = mybir.dt.bfloat16
AF = mybir.ActivationFunctionType
ALU = mybir.AluOpType

NCORES = 8
D = 1024
H = 16
HD = 64
HPC = H // NCORES          # heads per core = 2
DPC = HPC * HD             # embed dims per core = 128
EPS = 1e-5


class _OneTableBacc(bacc.Bacc):
    """Bacc whose activation-table pass may only pick the ln+exp+identity
    set. Every ACT func this kernel uses lives in that one set, so exactly
    one table load is emitted and phase-1 LN work can interleave with the
    softmax Exp stream with no table reloads."""

    _TABLE = "natural_log_exp_and_others"

    def insert_act_table_loads(self):
        from concourse.hw_specs import get_activation_tables

        all_tables = get_activation_tables(self.m.arch)
        assert self._TABLE in all_tables, f"{self._TABLE} missing"
        keep = all_tables[self._TABLE]
        # preserve list order (set ids are positional); make my funcs
        # resolvable only via the one combined table
        tables = [
            (k, v if k == self._TABLE else v - keep)
            for k, v in all_tables.items()
        ]
        import bass_rust as _bass_rust

        _bass_rust.insert_act_table_loads(self, tables)


def build_nc(B, S, affine):
    """Build the SPMD Bass program for one core (same program, 8 cores)."""
    T = B * S                      # total token columns
    NTB = T // 128                 # 128-token blocks
    QC = S // 512                  # q-chunks per batch
    KB = S // 128                  # k-blocks per batch
    KCH = D // 128                 # contraction chunks (8)
    SCALE = 1.0 / np.sqrt(HD)

    nc = _OneTableBacc(
        "TRN2",
        target_bir_lowering=False,
        debug=False,
        enable_asserts=True,
        num_devices=NCORES,
    )

    xT = nc.dram_tensor("xT", [D, T], BF16, kind="ExternalInput").ap()
    wq = nc.dram_tensor("wt_qkv", [D, 3 * DPC], BF16, kind="ExternalInput").ap()
    bqx = nc.dram_tensor("b_qkv_x", [128, 3 * DPC], F32, kind="ExternalInput").ap()
    wp = nc.dram_tensor("wt_proj", [DPC, D], BF16, kind="ExternalInput").ap()
    if affine:
        gb = nc.dram_tensor("c_gb", [128, 4, HD], F32, kind="ExternalInput").ap()
    outp = nc.dram_tensor("outp", [T, D], BF16, kind="ExternalOutput").ap()

    from contextlib import ExitStack

    with tile.TileContext(nc) as tc, ExitStack() as stack:
        const = stack.enter_context(tc.tile_pool(name="const", bufs=1))
        persist = stack.enter_context(tc.tile_pool(name="persist", bufs=1))

        # whole x^T resident in SBUF; the first token-chunk's DMA is issued
        # before the weights so the first qkv matmuls start ASAP
        xt_all = const.tile([128, KCH, T], BF16, tag="xt")
        nc.sync.dma_start(
            out=xt_all[:, :, 0:512],
            in_=xT.rearrange("(c p) t -> p c t", p=128)[:, :, 0:512],
        )
        wq_sb = const.tile([128, KCH, 3 * DPC], BF16, tag="wq")
        nc.sync.dma_start(
            out=wq_sb, in_=wq.rearrange("(c p) n -> p c n", p=128)
        )
        bqx_sb = const.tile([128, 3 * DPC], F32, tag="bqx")
        nc.sync.dma_start(out=bqx_sb, in_=bqx)
        for n in range(1, T // 512):
            nc.sync.dma_start(
                out=xt_all[:, :, n * 512 : (n + 1) * 512],
                in_=xT.rearrange("(c p) t -> p c t", p=128)[
                    :, :, n * 512 : (n + 1) * 512
                ],
            )
        wp_sb = const.tile([DPC, D], BF16, tag="wp")
        nc.sync.dma_start(out=wp_sb, in_=wp)
        eps_sb = const.tile([128, 1], F32, tag="eps")
        nc.vector.memset(eps_sb, EPS)

        if affine:
            gb_sb = const.tile([128, 4, HD], F32, tag="gb")
            nc.sync.dma_start(out=gb_sb, in_=gb)

        # persistent intermediates
        qT = persist.tile([128, T], BF16, tag="qT")     # [2h*64, tok] LN'd q^T
        kT = persist.tile([128, T], BF16, tag="kT")
        vO = persist.tile([128, HPC, NTB, HD + 1], BF16, tag="vO")
        aT = persist.tile([128, T], BF16, tag="aT")     # attention out^T
        nc.vector.memset(vO[:, :, :, HD : HD + 1], 1.0)

        # ---------------- Phase 1 emitter: qkv + LayerNorm + transpose ---
        stage1 = stack.enter_context(tc.tile_pool(name="stage1", bufs=8))
        stats_pool = stack.enter_context(tc.tile_pool(name="stats", bufs=6))

        def emit_qkv(tb, ps):
            """qkv matmuls into PSUM; stage q,k to SBUF bf16 and v to vO so
            the PSUM tile frees quickly. The qkv bias rides the staging
            copies (tensor_add with the host-broadcast bias tile) instead
            of costing a PE matmul. Returns the staged q,k tile."""
            for k in range(KCH):
                nc.tensor.matmul(
                    ps,
                    lhsT=xt_all[:, k, tb * 128 : (tb + 1) * 128],
                    rhs=wq_sb[:, k, :],
                    start=(k == 0),
                    stop=(k == KCH - 1),
                )
            qksb = stage1.tile([128, 4, HD], BF16, tag="qksb")
            nc.vector.tensor_add(
                qksb,
                ps[:, 0 : 2 * DPC].rearrange("p (g d) -> p g d", d=HD),
                bqx_sb[:, 0 : 2 * DPC].rearrange("p (g d) -> p g d", d=HD),
            )
            nc.vector.tensor_add(
                vO[:, :, tb, 0:HD],
                ps[:, 2 * DPC :].rearrange("p (h d) -> p h d", d=HD),
                bqx_sb[:, 2 * DPC :].rearrange("p (h d) -> p h d", d=HD),
            )
            return qksb

        def emit_ln(tb, qksb):
            """LayerNorm stats+apply from the SBUF staging, then DMA-xbar
            transpose into qT/kT."""
            st = stats_pool.tile([128, 4, 6], F32, tag="st")
            mv = stats_pool.tile([128, 4, 2], F32, tag="mv")
            for g in range(4):
                nc.vector.bn_stats(out=st[:, g, :], in_=qksb[:, g, :])
                nc.vector.bn_aggr(out=mv[:, g, :], in_=st[:, g, :])
            # rstd = (var+eps)^-1/2 as Exp(-0.5*Ln(var+eps)) — keeps every
            # ACT func inside the single ln+exp+identity table set
            lnv = stats_pool.tile([128, 4], F32, tag="lnv")
            nc.scalar.activation(
                out=lnv, in_=mv[:, :, 1], func=AF.Ln, bias=eps_sb
            )
            rstd = stats_pool.tile([128, 4], F32, tag="rstd")
            nc.scalar.activation(out=rstd, in_=lnv, func=AF.Exp, scale=-0.5)
            # nmr = -mu * rstd (bias for the ACT-side LN apply)
            nmr = stats_pool.tile([128, 4], F32, tag="nmr")
            nc.vector.scalar_tensor_tensor(
                out=nmr,
                in0=mv[:, :, 0],
                scalar=-1.0,
                in1=rstd,
                op0=ALU.mult,
                op1=ALU.mult,
            )
            qn = stage1.tile([128, 128], BF16, tag="qn")
            kn = stage1.tile([128, 128], BF16, tag="kn")
            for g in range(4):
                dst = qn if g < 2 else kn
                dsl = dst[:, (g % 2) * HD : (g % 2 + 1) * HD]
                if g < 2:
                    # q groups on ACT: (x - mu)*rstd == x*rstd + (-mu*rstd)
                    nc.scalar.activation(
                        out=dsl,
                        in_=qksb[:, g, :],
                        func=AF.Identity,
                        scale=rstd[:, g : g + 1],
                        bias=nmr[:, g : g + 1],
                    )
                else:
                    # k groups on DVE (bf16 in/out: 2x DVE mode)
                    nc.vector.tensor_scalar(
                        out=dsl,
                        in0=qksb[:, g, :],
                        scalar1=mv[:, g, 0:1],
                        scalar2=rstd[:, g : g + 1],
                        op0=ALU.subtract,
                        op1=ALU.mult,
                    )
                if affine:
                    nc.vector.tensor_mul(dsl, dsl, gb_sb[:, 2 * (g // 2), :])
                    nc.vector.tensor_add(
                        dsl, dsl, gb_sb[:, 2 * (g // 2) + 1, :]
                    )
            ts = slice(tb * 128, (tb + 1) * 128)
            nc.sync.dma_start_transpose(out=qT[:, ts], in_=qn)
            nc.sync.dma_start_transpose(out=kT[:, ts], in_=kn)

        # Single PSUM layout for both phases: the phase-1 qkv tiles ride
        # the projection pool's banks (temporally disjoint: projections
        # only start after the last qkv block), so batch 0's attention
        # chunks can be emitted in the middle of phase 1 — the PE-bound
        # qkv stream and the ACT-bound softmax stream overlap.
        with (
            tc.tile_pool(name="epi_ps", bufs=2, space="PSUM") as epi_ps,
            tc.tile_pool(name="o_ps", bufs=1, space="PSUM") as o_ps,
            tc.tile_pool(name="sc_ps", bufs=2, space="PSUM") as sc_ps,
            tc.tile_pool(name="exps", bufs=6) as exps,
            tc.tile_pool(name="stage2", bufs=4) as stage2,
            tc.tile_pool(name="ostage", bufs=3) as ostage,
        ):
            ooms = {}
            avs = {}
            pend = {}

            def emit_attnv(ci, kb, ex):
                b, _ = divmod(ci, QC)
                gkb = b * KB + kb
                oom = ooms[ci]
                for h in range(HPC):
                    nc.tensor.matmul(
                        oom[:, h, :],
                        lhsT=vO[:, h, gkb, :],
                        rhs=ex[:, h, :],
                        start=(kb == 0),
                        stop=(kb == KB - 1),
                    )

            def attn_kbs(ci, kbs):
                b, qc = divmod(ci, QC)
                cols = slice(b * S + qc * 512, b * S + (qc + 1) * 512)
                if ci not in ooms:
                    ooms[ci] = o_ps.tile(
                        [HD + 1, HPC, 512], F32, tag="o", name="oom"
                    )
                    pend[ci] = []
                for kb in kbs:
                    gkb = b * KB + kb
                    ks = slice(gkb * 128, (gkb + 1) * 128)
                    # two heads' score matmuls live at partition bases
                    # 0/64 -> disjoint PE row groups run concurrently;
                    # one 1024-wide exp covers both heads
                    scp = sc_ps.tile(
                        [128, HPC, 512], F32, tag="s", name="scp"
                    )
                    for h in range(HPC):
                        hp = slice(h * HD, (h + 1) * HD)
                        nc.tensor.matmul(
                            scp[:, h, :],
                            lhsT=kT[hp, ks],
                            rhs=qT[hp, cols],
                            start=True,
                            stop=True,
                        )
                    ex = exps.tile(
                        [128, HPC, 512], BF16, tag="ex", name="ex"
                    )
                    nc.scalar.activation(
                        out=ex, in_=scp, func=AF.Exp, scale=SCALE
                    )
                    # lag attnv by one kb so PE never convoys behind the
                    # chunk-start oom handoff: the next scores are already
                    # issued before the first attnv can stall
                    pend[ci].append((kb, ex))
                    if len(pend[ci]) > 1:
                        emit_attnv(ci, *pend[ci].pop(0))

            def evict_chunk(ci):
                # flush lagged attnvs, then evacuate the PSUM accumulator
                # early (raw attention sums + denominator row) so the next
                # chunk's attnv can reuse the single oom buffer without
                # waiting on the normalize chain
                for kb, ex in pend.pop(ci):
                    emit_attnv(ci, kb, ex)
                oom = ooms.pop(ci)
                dn = stage2.tile([1, HPC, 512], F32, tag="dn", name="dn")
                nc.vector.tensor_copy(out=dn, in_=oom[HD : HD + 1, :, :])
                # per-head staging at matching partition offsets (the BIR
                # verifier requires SBUF operands on identical partitions;
                # only the PSUM side may shift)
                av = stage2.tile([128, 512], F32, tag="av", name="av")
                for h in range(HPC):
                    nc.vector.tensor_copy(
                        out=av[h * HD : (h + 1) * HD, :], in_=oom[0:HD, h, :]
                    )
                avs[ci] = (dn, av)

            def norm_proj_chunk(ci):
                b, qc = divmod(ci, QC)
                cols = slice(b * S + qc * 512, b * S + (qc + 1) * 512)
                dn, av = avs.pop(ci)
                # 1/denominator on DVE (fast 18-bit custom op), broadcast
                # across partitions on the idle gpsimd engine (ACT stays
                # pure-Exp: no activation-table reloads in the phase)
                rc = stage2.tile([1, HPC, 512], F32, tag="rc", name="rc")
                nc.vector.reciprocal_approx_fast(out=rc, in_=dn)
                rbs = stage2.tile(
                    [128, HPC, 512], F32, tag="rbs", name="rbs"
                )
                nc.gpsimd.partition_broadcast(rbs, rc)
                for h in range(HPC):
                    hp = slice(h * HD, (h + 1) * HD)
                    nc.vector.tensor_mul(
                        aT[hp, cols], av[hp, :], rbs[hp, h, :]
                    )
                # fused partial projection for the 4 token blocks of this
                # q-chunk; PSUM evicted to SBUF bf16 (projection bias is
                # added on the host)
                for tbl in range(4):
                    tb = ci * 4 + tbl
                    rows = slice(tb * 128, (tb + 1) * 128)
                    ob = ostage.tile([128, D], BF16, tag="ob")
                    for nn in range(D // 512):
                        pps = epi_ps.tile(
                            [128, 512], F32, tag="pps", name="pps"
                        )
                        nc.tensor.matmul(
                            pps,
                            lhsT=aT[:, rows],
                            rhs=wp_sb[:, nn * 512 : (nn + 1) * 512],
                            start=True,
                            stop=True,
                        )
                        nc.vector.tensor_copy(
                            out=ob[:, nn * 512 : (nn + 1) * 512], in_=pps
                        )
                    nc.sync.dma_start(out=outp[rows, :], in_=ob)

            def emit_p1(tb):
                ps = epi_ps.tile([128, 512], F32, tag="pps", name="qkv_ps")
                emit_ln(tb, emit_qkv(tb, ps[:, 0 : 3 * DPC]))

            # Phase 1 prologue: batch 0's 16 token blocks
            for tb in range(NTB // 2):
                emit_p1(tb)
            # Phase 1 tail interleaved with batch 0's first two attention
            # chunks (their q/k/v are complete): PE alternates qkv and
            # score/attnv matmuls while ACT runs 32 softmax Exps early
            for i in range(NTB // 2):
                emit_p1(NTB // 2 + i)
                ci, half = divmod(i, QC * 2)
                attn_kbs(ci, (2 * half, 2 * half + 1))
                if half == QC * 2 - 1:
                    evict_chunk(ci)

            # steady state: chunk ci's attention brackets an older chunk's
            # normalize+projection (emitted mid-chunk so its PE matmuls and
            # DVE casts drain before chunk ci's oom eviction needs the DVE
            # queue); the PSUM accumulator is evicted immediately after each
            # chunk's last attnv so the single oom buffer hands off fast
            NCHUNK = B * QC
            norm_pending = [0, 1]
            for ci in range(2, NCHUNK):
                attn_kbs(ci, range(0, KB // 2))
                norm_proj_chunk(norm_pending.pop(0))
                attn_kbs(ci, range(KB // 2, KB))
                evict_chunk(ci)
                norm_pending.append(ci)
            for ci in norm_pending:
                norm_proj_chunk(ci)

    nc.compile()
    return nc


def make_in_maps(x, w_qkv, b_qkv, w_proj, q_gamma, q_beta, k_gamma, k_beta,
                 affine):
    B, S, _ = x.shape
    T = B * S
    xT = np.ascontiguousarray(x.reshape(T, D).T)
    import ml_dtypes
    bf = ml_dtypes.bfloat16
    in_maps = []
    for c in range(NCORES):
        rs = slice(c * DPC, (c + 1) * DPC)
        w_slice = np.concatenate(
            [w_qkv[rs], w_qkv[D:2 * D][rs.start:rs.stop], w_qkv[2 * D:][rs.start:rs.stop]],
            axis=0,
        )  # [384, 1024]
        b_slice = np.concatenate(
            [b_qkv[rs], b_qkv[D:2 * D][rs.start:rs.stop], b_qkv[2 * D:][rs.start:rs.stop]]
        )[None, :]  # [1, 384]
        m = {
            "xT": xT.astype(bf),
            "wt_qkv": np.ascontiguousarray(w_slice.T).astype(bf),
            "b_qkv_x": np.ascontiguousarray(
                np.broadcast_to(b_slice, (128, 3 * DPC))
            ).astype(np.float32),
            "wt_proj": np.ascontiguousarray(w_proj[:, rs].T).astype(bf),
        }
        if affine:
            gb = np.stack([q_gamma, q_beta, k_gamma, k_beta])  # [4, 64]
            m["c_gb"] = np.ascontiguousarray(
                np.broadcast_to(gb[None], (128, 4, HD)).astype(np.float32)
            )
        in_maps.append(m)
    return in_maps


_NC_CACHE = {}

LAST_RESULTS = None


def kernel(x, w_qkv, b_qkv, w_proj, b_proj, q_gamma, q_beta, k_gamma, k_beta,
           **unused):
    global LAST_RESULTS
    x = np.asarray(x, np.float32)
    w_qkv = np.asarray(w_qkv, np.float32)
    b_qkv = np.asarray(b_qkv, np.float32)
    w_proj = np.asarray(w_proj, np.float32)
    b_proj = np.asarray(b_proj, np.float32)
    q_gamma = np.asarray(q_gamma, np.float32)
    q_beta = np.asarray(q_beta, np.float32)
    k_gamma = np.asarray(k_gamma, np.float32)
    k_beta = np.asarray(k_beta, np.float32)

    B, S, _ = x.shape
    affine = not (
        np.all(q_gamma == 1) and np.all(k_gamma == 1)
        and np.all(q_beta == 0) and np.all(k_beta == 0)
    )
    key = (B, S, affine)
    if key not in _NC_CACHE:
        _NC_CACHE[key] = build_nc(B, S, affine)
    nc = _NC_CACHE[key]

    in_maps = make_in_maps(
        x, w_qkv, b_qkv, w_proj, q_gamma, q_beta, k_gamma, k_beta, affine
    )
    trace = bool(int(os.environ.get("BASS_KERNEL_TRACE", "0")))
    res = run_bass_kernel_spmd(
        nc, in_maps, core_ids=list(range(NCORES)), trace=trace
    )
    LAST_RESULTS = res
    acc = np.zeros((B * S, D), np.float32)
    for r in res.results:
        acc += np.asarray(r["outp"], np.float32)
    acc += b_proj[None, :]
    return acc.reshape(B, S, D)
